# revision 3
# baseline (speedup 1.0000x reference)
"""Trainium2 Bass kernel for nn_EnhancedWaveletTransform2D.

Math (exact algebraic reductions of the reference):
  - wavedec2/waverec2 round trip == identity  ->  x_wave = x
  - conv(x*a) = a*conv(x) (depthwise), and InstanceNorm(affine=False) makes
    both the conv bias refine_b and any per-channel scale fold into the
    final affine:
        u   = depthwise_conv3x3(x)            (no bias, no attention scale)
        S_c = 1 / sqrt(var(u_c) + eps/a_c^2) ~= 1 / sqrt(var(u_c) + 4*eps)
        T_c = -mean(u_c) * S_c
        out = leaky_relu(u * S + T, 0.01)
    (a = sigmoid(O(1e-2)) ~= 0.5, so eps/a^2 ~= 4*eps to ~2e-6 rel err.)

Implementation v2 (vs v1 baseline at 146.5us):
  - x, u, y all bf16 (halves HBM traffic both directions; ~0.7% rel err,
    tolerance is 2e-2).  Whole x block resident in SBUF: no halo windows,
    no pad columns -- edge zero-padding comes free from clipped APs
    (shifted in/out ranges).
  - 9 conv taps split across engines by cost-model rates
    (PE 0.42 ns/elem, DVE 1.07, Pool 1.39-1.47, ACT 0.92):
      PE:   6 off-row taps (dh != 0) as bf16 diagonal matmuls -> PSUM
      Pool: center tap (tensor_scalar_mul) + alternating (0,-1) -> SBUF acc
      DVE:  (0,+1) tap + alternating (0,-1) -> same SBUF acc, then the
            merge/evac stt: u = acc + psum (accum_out = sum(u))
      ACT:  Square pass (accum_out = sum(u^2)), final fused
            lrelu(S*u+T) via Lrelu with per-partition scale/bias
  - final pass of the LAST block is on the post-stats critical path, so it
    is split ACT/DVE (DVE uses lrelu(v)=max(S*u+T, slope*S*u), dropping the
    slope*T term: ~2e-4 rel err).
"""
import os
import numpy as np
import ml_dtypes

import concourse.tile as tile
from concourse import bacc, mybir
from concourse.bass_utils import run_bass_kernel_spmd

F32 = mybir.dt.float32
BF16 = mybir.dt.bfloat16
AF = mybir.ActivationFunctionType
OP = mybir.AluOpType

C = 256
H = W = 128
HW = H * W
NBLK = 2          # channel blocks of 128
P = 128           # partitions
TROWS = 16        # rows per psum tile
NT = H // TROWS   # 8 tiles per block
SEG = 4           # rows per matmul (<=512 free)
NSEG = TROWS // SEG
EPS = 1e-5
SLOPE = 0.01
TAPS = [(di, dj) for di in (-1, 0, 1) for dj in (-1, 0, 1)]
PE_TAPS = [(di, dj) for (di, dj) in TAPS if di != 0]   # 6 taps
# tap index in the 3x3 kernel (row-major) for weight lookup
TIDX = {t: i for i, t in enumerate(TAPS)}


def _rows(r0, r1, dh):
    """Clip row range [r0,r1) shifted by dh to the image; return (in0,in1,out0,out1)."""
    i0, i1 = max(0, r0 + dh), min(H, r1 + dh)
    return i0, i1, i0 - dh, i1 - dh


def _cols(dw):
    """Clipped col ranges for horizontal shift dw: (cin0,cin1,cout0,cout1)."""
    ci0, ci1 = max(0, dw), min(W, W + dw)
    return ci0, ci1, ci0 - dw, ci1 - dw


def _iteration(nc, pools, consts, skip=()):
    xsb_pool, u_pool, acc_pool, dump_pool, small, psum_pool = pools
    diag_sb, wcol_sb, eps4_sb, x_d, y_d = consts

    su = [small.tile([P, NT], F32, tag=f"su{b}", name=f"su{b}") for b in range(NBLK)]
    ssq = [small.tile([P, NT], F32, tag=f"ssq{b}", name=f"ssq{b}") for b in range(NBLK)]
    S_sb = small.tile([P, NBLK], F32, tag="Ssb", name="Ssb")
    Sp_sb = small.tile([P, NBLK], F32, tag="Spsb", name="Spsb")
    T_sb = small.tile([P, NBLK], F32, tag="Tsb", name="Tsb")
    st_tmp = small.tile([P, 4], F32, tag="sttmp", name="sttmp")

    xsb = []
    u_blk = []
    for b in range(NBLK):
        xsb.append(xsb_pool.tile([P, H, W], BF16, tag=f"x{b}", name=f"x{b}"))
        u_blk.append(u_pool.tile([P, HW], BF16, tag=f"u{b}", name=f"u{b}"))

    # ---------------- input DMA (4 chunks of 32 rows per block) ----------
    if "indma" not in skip:
        for b in range(NBLK):
            for ch in range(4):
                r0 = ch * 32
                nc.sync.dma_start(
                    out=xsb[b][:, r0 : r0 + 32, :],
                    in_=x_d[b, :, r0 : r0 + 32, :],
                )

    def w_ap(b, t):
        i = TIDX[t]
        return wcol_sb[:, b * 9 + i : b * 9 + i + 1]

    # ---------------- conv + stats streaming ----------------
    for b in range(NBLK):
        for ti in range(NT):
            r0 = ti * TROWS
            ps = psum_pool.tile([P, TROWS * W], F32, tag="ps", name="ps")
            ps3 = ps.rearrange("p (r c) -> p r c", r=TROWS)

            # --- PE: 6 off-row taps as bf16 diagonal matmuls ---
            if "pe" not in skip:
                for s in range(NSEG):
                    s0 = r0 + s * SEG
                    s1 = s0 + SEG
                    # first tap must fully cover the seg region (psum init)
                    first = (1, 0) if s1 < H else (-1, 0)
                    taps = [first] + [t for t in PE_TAPS if t != first]
                    for k, (dh, dw) in enumerate(taps):
                        ri0, ri1, ro0, ro1 = _rows(s0, s1, dh)
                        ci0, ci1, co0, co1 = _cols(dw)
                        nc.tensor.matmul(
                            out=ps3[:, ro0 - r0 : ro1 - r0, co0:co1],
                            lhsT=diag_sb[b][:, TIDX[(dh, dw)], :],
                            rhs=xsb[b][:, ri0:ri1, ci0:ci1],
                            start=(k == 0),
                            stop=(k == len(taps) - 1),
                        )

            # --- Pool: center tap + alternating (0,-1) into SBUF acc ---
            acc = acc_pool.tile([P, TROWS, W], BF16, tag="acc", name="acc")
            if "pool" not in skip:
                nc.gpsimd.tensor_scalar_mul(
                    out=acc,
                    in0=xsb[b][:, r0 : r0 + TROWS, :],
                    scalar1=w_ap(b, (0, 0)),
                )
            else:
                nc.vector.memset(acc, 0.0)
            pool_has_m1 = (ti % 2 == 0)
            if "pool" not in skip and pool_has_m1:
                ci0, ci1, co0, co1 = _cols(-1)
                nc.gpsimd.scalar_tensor_tensor(
                    out=acc[:, :, co0:co1],
                    in0=xsb[b][:, r0 : r0 + TROWS, ci0:ci1],
                    scalar=w_ap(b, (0, -1)),
                    in1=acc[:, :, co0:co1],
                    op0=OP.mult,
                    op1=OP.add,
                )

            # --- DVE taps into the same acc ---
            if "dve" not in skip:
                dtaps = [(0, 1)] + ([] if pool_has_m1 else [(0, -1)])
                for dh, dw in dtaps:
                    ci0, ci1, co0, co1 = _cols(dw)
                    nc.vector.scalar_tensor_tensor(
                        out=acc[:, :, co0:co1],
                        in0=xsb[b][:, r0 : r0 + TROWS, ci0:ci1],
                        scalar=w_ap(b, (dh, dw)),
                        in1=acc[:, :, co0:co1],
                        op0=OP.mult,
                        op1=OP.add,
                    )

            # --- DVE merge/evac: u = acc + psum, accum_out = sum(u) ---
            u3 = u_blk[b][:, ti * TROWS * W : (ti + 1) * TROWS * W].rearrange(
                "p (r c) -> p r c", r=TROWS
            )
            if "evac" not in skip:
                nc.vector.scalar_tensor_tensor(
                    out=u3,
                    in0=acc,
                    scalar=1.0,
                    in1=ps3,
                    op0=OP.mult,
                    op1=OP.add,
                    accum_out=su[b][:, ti : ti + 1],
                )

            # --- ACT: sum(u^2) via Square + accum ---
            if "sq" not in skip:
                dump = dump_pool.tile([P, TROWS * W], BF16, tag="dump", name="dump")
                nc.scalar.activation(
                    out=dump,
                    in_=u_blk[b][:, ti * TROWS * W : (ti + 1) * TROWS * W],
                    func=AF.Square,
                    accum_out=ssq[b][:, ti : ti + 1],
                )

        # ---------------- per-block affine S, T ----------------
        if "stats" in skip:
            continue
        mean = st_tmp[:, 0:1]
        sumsq = st_tmp[:, 1:2]
        var = st_tmp[:, 2:3]
        sd = st_tmp[:, 3:4]
        nc.vector.reduce_sum(out=mean, in_=su[b], axis=mybir.AxisListType.X)
        nc.vector.tensor_scalar_mul(out=mean, in0=mean, scalar1=1.0 / HW)
        nc.vector.reduce_sum(out=sumsq, in_=ssq[b], axis=mybir.AxisListType.X)
        nc.vector.tensor_mul(out=var, in0=mean, in1=mean)
        nc.vector.scalar_tensor_tensor(
            out=var, in0=sumsq, scalar=1.0 / HW, in1=var,
            op0=OP.mult, op1=OP.subtract,
        )
        nc.scalar.activation(out=sd, in_=var, func=AF.Sqrt, bias=eps4_sb)
        nc.vector.reciprocal(out=S_sb[:, b : b + 1], in_=sd)
        nc.vector.scalar_tensor_tensor(
            out=T_sb[:, b : b + 1], in0=mean, scalar=-1.0, in1=S_sb[:, b : b + 1],
            op0=OP.mult, op1=OP.mult,
        )
        nc.vector.tensor_scalar_mul(
            out=Sp_sb[:, b : b + 1], in0=S_sb[:, b : b + 1], scalar1=SLOPE
        )

    # ---------------- final normalize + leaky + store ----------------
    # Non-last blocks: all on ACT (overlaps next block's tap phase).
    # Last block is the exposed tail -> split ACT/DVE.
    for b in range(NBLK):
        last = b == NBLK - 1
        nchunk = 8  # 2048 px per final chunk
        csz = HW // nchunk
        for ci in range(nchunk):
            sl = slice(ci * csz, (ci + 1) * csz)
            uc = u_blk[b][:, sl]
            on_dve = last and (ci % 3 == 2)  # ~1/3 of tail chunks on DVE
            if "final" not in skip:
                if on_dve:
                    v = dump_pool.tile([P, csz], BF16, tag="fv", name="fv")
                    nc.vector.tensor_scalar(
                        out=v, in0=uc,
                        scalar1=S_sb[:, b : b + 1], scalar2=T_sb[:, b : b + 1],
                        op0=OP.mult, op1=OP.add,
                    )
                    nc.vector.scalar_tensor_tensor(
                        out=uc, in0=uc, scalar=Sp_sb[:, b : b + 1], in1=v,
                        op0=OP.mult, op1=OP.max,
                    )
                else:
                    nc.scalar.activation(
                        out=uc, in_=uc, func=AF.Lrelu,
                        bias=T_sb[:, b : b + 1], scale=S_sb[:, b : b + 1],
                        alpha=SLOPE,
                    )
            if "outdma" not in skip:
                rows = csz // W
                r0 = ci * rows
                nc.sync.dma_start(
                    out=y_d[b, :, r0 : r0 + rows, :],
                    in_=uc.rearrange("p (r c) -> p r c", r=rows),
                )


def build_nc(repeat=1, skip=()):
    nc = bacc.Bacc("TRN2", target_bir_lowering=False)
    x_d = nc.declare_dram_parameter("x", [NBLK, P, H, W], BF16, isOutput=False)
    diag_d = nc.declare_dram_parameter("diag", [NBLK, P, 9, P], BF16, isOutput=False)
    wcol_d = nc.declare_dram_parameter("wcol", [P, NBLK * 9], F32, isOutput=False)
    y_d = nc.declare_dram_parameter("y", [NBLK, P, H, W], BF16, isOutput=True)

    with tile.TileContext(nc) as tc:
        with (
            tc.tile_pool(name="xsb", bufs=1) as xsb_pool,
            tc.tile_pool(name="ublk", bufs=1) as u_pool,
            tc.tile_pool(name="acc", bufs=3) as acc_pool,
            tc.tile_pool(name="dump", bufs=2) as dump_pool,
            tc.tile_pool(name="small", bufs=1) as small,
            tc.tile_pool(name="psum", bufs=2, space="PSUM") as psum_pool,
        ):
            diag_sb = [
                small.tile([P, 9, P], BF16, tag=f"diag{b}", name=f"diag{b}")
                for b in range(NBLK)
            ]
            wcol_sb = small.tile([P, NBLK * 9], F32, tag="wcol", name="wcol")
            eps4_sb = small.tile([P, 1], F32, tag="eps4", name="eps4")
            nc.vector.memset(eps4_sb, 4.0 * EPS)
            for b in range(NBLK):
                nc.gpsimd.dma_start(out=diag_sb[b], in_=diag_d[b])
            nc.gpsimd.dma_start(out=wcol_sb, in_=wcol_d[:])

            pools = (xsb_pool, u_pool, acc_pool, dump_pool, small, psum_pool)
            consts = (diag_sb, wcol_sb, eps4_sb, x_d, y_d)
            for _ in range(repeat):
                _iteration(nc, pools, consts, skip=skip)
    nc.compile()
    return nc


_NC_CACHE = {}


def _get_nc(repeat=1):
    if repeat not in _NC_CACHE:
        _NC_CACHE[repeat] = build_nc(repeat)
    return _NC_CACHE[repeat]


def make_in_maps(x, attn_w1, attn_w2, refine_w):
    """Host-side prep of per-core input maps (weights are tiny)."""
    B = x.shape[0]
    wt = refine_w.reshape(C, 9)                      # [256, 9] tap columns
    diag = np.zeros((NBLK, P, 9, P), np.float32)
    idx = np.arange(P)
    for b in range(NBLK):
        for t in range(9):
            diag[b, idx, t, idx] = wt[b * P : (b + 1) * P, t]
    wcol = np.empty((P, NBLK * 9), np.float32)
    for b in range(NBLK):
        wcol[:, b * 9 : (b + 1) * 9] = wt[b * P : (b + 1) * P, :]
    shared = {
        "diag": diag.astype(ml_dtypes.bfloat16),
        "wcol": wcol,
    }
    xb = x.astype(ml_dtypes.bfloat16)
    return [{"x": xb[i].reshape(NBLK, P, H, W), **shared} for i in range(B)]


def run_nc(nc, in_maps):
    return run_bass_kernel_spmd(nc, in_maps, core_ids=list(range(len(in_maps))))


def kernel(x, attn_w1, attn_w2, refine_w, refine_b):
    x = np.asarray(x, dtype=np.float32)
    refine_w = np.asarray(refine_w, dtype=np.float32)
    B = x.shape[0]

    in_maps = make_in_maps(x, attn_w1, attn_w2, refine_w)
    nc = _get_nc(int(os.environ.get("KREPEAT", "1")))
    res = run_nc(nc, in_maps)
    out = np.stack(
        [np.asarray(res.results[i]["y"]).reshape(C, H, W) for i in range(B)]
    )
    return out.astype(np.float32)
